# revision 13
# baseline (speedup 1.0000x reference)
"""Trainium2 Bass kernel for quantized 3x3 conv2d (stride 1, pad 1).

Reference computes: conv2d(quant16(x), quant16(w)) where quant16 rounds to
signed 16-bit fixed point with 12 fractional bits (round-half-even, /4096).

Strategy (per core, data-parallel over batch: 4 images/core on 8 cores):
  - Tolerance is rel_err < 2e-2; a single fp16 term is plenty (measured
    2.1e-4 on the real inputs): xh = fp16(x*4096) keeps an 11-bit
    significand and rw = round(w*4096) is fp16-exact (|rw| < 2048). Both
    conversions happen on the host; input DMA bytes are halved. The output
    is stored as fp16 too (adds ~5e-4 rel err, total ~7e-4), halving the
    store traffic so the tail never backs up on HBM write bandwidth.
  - 3x3 conv = 9 shifted matmuls accumulating in PSUM over a zero-padded
    58x58 image laid out [Cin=128 partitions, 58*58]. Contraction dim =
    partition dim = Cin = 128. Cout=256 -> two 128-row output chunks.
  - Work is cut into 32 half-rounds of 2 row-groups (2 PSUM banks, 784 px)
    cycling over four 2-bank PSUM tiles. Each half-round: 18 matmuls
    (taps outer, 2 share a stationary), then ONE strided 2-bank eviction
    (alternating ScalarE/VectorE, applying the 2^-24 fixed-point scale and
    the f32->f16 convert) and ONE 200KB store (alternating HW-DGE rings).
    Separate tiles keep evictions off the matmul critical path (the
    dependency tracker is whole-tile), and the small evict+store tail after
    the last matmul hides under the fixed ~8us semaphore-cleanup epilogue.
  - Ramp: ~26 dummy matmuls on a zeroed tile warm the PE HAM clock gate
    (1.2 -> 2.4 GHz); the first real matmuls' DMA deps are minimized
    (w ch0 split per-tap-triple on the Scalar ring, x rows 0-8 first on
    the Sync ring) and half-round 0 runs taps 0-2 across both groups
    before taps 3-8 so compute never outruns the staged chunks.
"""

import numpy as np

B, CIN, COUT, H, W = 32, 128, 256, 56, 56
NCORES = 8
BL = B // NCORES          # images per core
HP = H + 2                # padded height/width (58)
NPIX = H * W              # 3136
NPAD = HP * HP            # 3364
SCALE = 4096.0
OSCALE = 1.0 / (SCALE * SCALE)
GROUP_ROWS = 7            # output rows per PSUM bank
GRP_PIX = GROUP_ROWS * W  # 392
HR_PIX = 2 * GRP_PIX      # 784 px per half-round (2 banks)
ROUND_PIX = 4 * GRP_PIX   # 1568 px per (ch, half) round
NWARM = 26                # dummy matmuls to warm the PE clock gate

_cache = {}


def _build():
    import concourse.bacc as bacc
    import concourse.mybir as mybir
    import concourse.tile as tile

    f32, f16 = mybir.dt.float32, mybir.dt.float16
    Copy = mybir.ActivationFunctionType.Copy

    nc = bacc.Bacc("TRN2", target_bir_lowering=False)
    # x arrives zero-padded to 58x58 and pre-quantized to fp16 on the host
    x_in = nc.dram_tensor("x", [BL, CIN, NPAD], f16, kind="ExternalInput")
    w_in = nc.dram_tensor("w", [CIN, 9 * COUT], f16, kind="ExternalInput")
    out = nc.dram_tensor("out", [BL, COUT, NPIX], f16, kind="ExternalOutput")

    with tile.TileContext(nc) as tc:
        with (
            tc.tile_pool(name="fixed", bufs=1) as fx,
            tc.tile_pool(name="psum", bufs=1, space="PSUM") as pp,
        ):
            xhs = [fx.tile([CIN, NPAD], f16, name=f"xh{i}") for i in range(2)]
            osbs = [fx.tile([128, HR_PIX], f16, name=f"osb{i}") for i in range(4)]
            pq = [pp.tile([128, 2 * 512], f32, name=f"pq{i}") for i in range(4)]
            w16 = fx.tile([CIN, 9 * COUT], f16)
            dummy = fx.tile([128, 256], f16, name="dummy")

            # ---- PE warm-up: keep the HAM activity window busy from engine
            # start so the real matmul stream begins at 2.4 GHz.
            nc.gpsimd.memset(dummy[:], 0.0)
            for i in range(NWARM):
                nc.tensor.matmul(
                    pq[0][:, :128], dummy[:, :128], dummy[:, 128:],
                    start=True, stop=True,
                )

            def stage(b, r0, r1):
                lo, hi = r0 * HP, r1 * HP
                nc.sync.dma_start(out=xhs[b % 2][:, lo:hi], in_=x_in[b, :, lo:hi])

            # x chunks ride the Sync HW-DGE ring, w chunks the Scalar ring:
            # both first transfers issue concurrently.
            HW_COLS = 9 * 128  # 1152 columns per cout-half
            stage(0, 0, 9)
            nc.scalar.dma_start(out=w16[:, : 3 * 128], in_=w_in[:, : 3 * 128])
            stage(0, 9, 16)
            nc.scalar.dma_start(out=w16[:, 3 * 128 : 6 * 128], in_=w_in[:, 3 * 128 : 6 * 128])
            stage(0, 16, 30)
            nc.scalar.dma_start(out=w16[:, 6 * 128 : HW_COLS], in_=w_in[:, 6 * 128 : HW_COLS])
            stage(0, 30, HP)
            nc.scalar.dma_start(out=w16[:, HW_COLS:], in_=w_in[:, HW_COLS:])
            stage(1, 0, HP)

            hr = 0
            for b in range(BL):
                if b >= 2:
                    stage(b, 0, HP)
                xh3 = xhs[b % 2][:].rearrange("p (h w) -> p h w", h=HP)

                for ch in range(2):
                    for half in range(2):
                        for hi in range(2):
                            t = pq[hr % 4]
                            osb = osbs[hr % 4]
                            gs = (2 * hi, 2 * hi + 1)
                            if hr == 0:
                                # tap-triple major: each triple needs only one
                                # staged w chunk and the already-landed x rows,
                                # so the cold ramp never stalls on DMA
                                order = [
                                    (tap, s)
                                    for tri in range(3)
                                    for s in range(2)
                                    for tap in range(3 * tri, 3 * tri + 3)
                                ]
                            else:
                                order = [(tap, s) for tap in range(9) for s in range(2)]
                            for tap, s in order:
                                g = gs[s]
                                dh, dw = divmod(tap, 3)
                                wsl = w16[:, ch * HW_COLS + tap * 128 : ch * HW_COLS + tap * 128 + 128]
                                r0 = (half * 4 + g) * GROUP_ROWS
                                mv = xh3[:, r0 + dh : r0 + dh + GROUP_ROWS, dw : dw + W]
                                nc.tensor.matmul(
                                    t[:, s * 512 : s * 512 + GRP_PIX], wsl, mv,
                                    start=(tap == 0), stop=(tap == 8),
                                )
                            base = half * ROUND_PIX + 2 * hi * GRP_PIX
                            if hr == 31:
                                # final half-round: evict the two banks on
                                # both engines in parallel and store on both
                                # DGE rings so the tail is as short as
                                # possible before the fixed epilogue
                                nc.scalar.activation(
                                    osb[:, :GRP_PIX], t[:, :GRP_PIX], Copy, scale=OSCALE
                                )
                                nc.sync.dma_start(
                                    out=out[b, ch * 128 : (ch + 1) * 128, base : base + GRP_PIX],
                                    in_=osb[:, :GRP_PIX],
                                )
                                nc.vector.tensor_scalar_mul(
                                    osb[:, GRP_PIX:], t[:, 512 : 512 + GRP_PIX], OSCALE
                                )
                                nc.scalar.dma_start(
                                    out=out[b, ch * 128 : (ch + 1) * 128, base + GRP_PIX : base + HR_PIX],
                                    in_=osb[:, GRP_PIX:],
                                )
                            else:
                                # one strided 2-bank eviction (scale + f32->f16)
                                src = t[:].rearrange("p (g c) -> p g c", c=512)[:, :, :GRP_PIX]
                                dst = osb[:].rearrange("p (g c) -> p g c", c=GRP_PIX)
                                if hr % 2 == 0:
                                    nc.scalar.activation(dst, src, Copy, scale=OSCALE)
                                else:
                                    nc.vector.tensor_scalar_mul(dst, src, OSCALE)
                                dstap = out[b, ch * 128 : (ch + 1) * 128, base : base + HR_PIX]
                                if hr % 2 == 0:
                                    nc.sync.dma_start(out=dstap, in_=osb[:])
                                else:
                                    nc.scalar.dma_start(out=dstap, in_=osb[:])
                            hr += 1
    nc.compile()
    return nc


def _get_nc():
    if "nc" not in _cache:
        _cache["nc"] = _build()
    return _cache["nc"]


def _maybe_install_trace_bridge():
    """Optional: bridge antenv.axon_hooks so trace=True can capture NTFF."""
    import sys
    import types

    if "antenv.axon_hooks" in sys.modules:
        return
    try:
        from trn_agent_boot.trn_boot import _ntff_profile_via_ctypes

        hook = _ntff_profile_via_ctypes("/opt/axon/libaxon_pjrt.so")
        mod = types.ModuleType("antenv.axon_hooks")
        mod.get_axon_ntff_profile_hook = lambda: hook
        mod.set_axon_ntff_profile_hook = lambda h: None
        import antenv

        sys.modules["antenv.axon_hooks"] = mod
        antenv.axon_hooks = mod
    except Exception:
        pass


def kernel(**inputs):
    import os

    from concourse.bass_utils import run_bass_kernel_spmd

    x = np.asarray(inputs["x"], dtype=np.float32)
    weight = np.asarray(inputs["weight"], dtype=np.float32)
    assert x.shape == (B, CIN, H, W), x.shape
    assert weight.shape == (COUT, CIN, 3, 3), weight.shape

    # rw = round(w*4096) is an integer < 2048 -> exact in fp16.
    # [Cout, Cin, kh, kw] -> [Cin, (ch, kh kw, co128)] so each (ch, tap)
    # slice is a ready [K=ci, M=co] stationary operand, ch-major so the
    # kernel can stage the ch=0 half first.
    rw = np.rint(weight * np.float32(SCALE))
    w_r = np.ascontiguousarray(
        rw.reshape(2, 128, CIN, 9)
        .transpose(2, 0, 3, 1)
        .reshape(CIN, 9 * COUT)
        .astype(np.float16)
    )
    # xh = fp16(x*4096): the *4096 is exact in f32 (power of two), the fp16
    # cast is the only rounding. Zero-pad to 58x58 so every DMA is contiguous.
    xp = np.zeros((B, CIN, HP, HP), dtype=np.float16)
    xp[:, :, 1 : 1 + H, 1 : 1 + W] = (x * np.float32(SCALE)).astype(np.float16)
    xp = xp.reshape(B, CIN, NPAD)
    in_maps = [
        {"x": xp[i * BL : (i + 1) * BL], "w": w_r}
        for i in range(NCORES)
    ]

    trace = bool(int(os.environ.get("KERNEL_TRACE", "0")))
    if trace:
        _maybe_install_trace_bridge()
    nc = _get_nc()
    res = run_bass_kernel_spmd(nc, in_maps, core_ids=list(range(NCORES)), trace=trace)
    _cache["exec_time_ns"] = res.exec_time_ns
    _cache["res"] = res

    outs = [
        res.results[i]["out"].astype(np.float32).reshape(BL, COUT, H, W)
        for i in range(NCORES)
    ]
    return np.concatenate(outs, axis=0)


# revision 14
# speedup vs baseline: 1.0235x; 1.0235x over previous
"""Trainium2 Bass kernel for quantized 3x3 conv2d (stride 1, pad 1).

Reference computes: conv2d(quant16(x), quant16(w)) where quant16 rounds to
signed 16-bit fixed point with 12 fractional bits (round-half-even, /4096).

Strategy (per core, data-parallel over batch: 4 images/core on 8 cores):
  - Tolerance is rel_err < 2e-2; a single fp16 term is plenty (measured
    2.1e-4 on the real inputs): xh = fp16(x*4096) keeps an 11-bit
    significand and rw = round(w*4096) is fp16-exact (|rw| < 2048). Both
    conversions happen on the host; input DMA bytes are halved. The output
    is stored as fp16 too (adds ~5e-4 rel err, total ~7e-4), halving the
    store traffic so the tail never backs up on HBM write bandwidth.
  - 3x3 conv = 9 shifted matmuls accumulating in PSUM over a zero-padded
    58x58 image laid out [Cin=128 partitions, 58*58]. Contraction dim =
    partition dim = Cin = 128. Cout=256 -> two 128-row output chunks.
  - Work is cut into 32 half-rounds of 2 row-groups (2 PSUM banks, 784 px)
    cycling over four 2-bank PSUM tiles. Each half-round: 18 matmuls
    (taps outer, 2 share a stationary), then ONE strided 2-bank eviction
    (alternating ScalarE/VectorE, applying the 2^-24 fixed-point scale and
    the f32->f16 convert) and ONE 200KB store (alternating HW-DGE rings).
    Separate tiles keep evictions off the matmul critical path (the
    dependency tracker is whole-tile), and the small evict+store tail after
    the last matmul hides under the fixed ~8us semaphore-cleanup epilogue.
  - Ramp: ~26 dummy matmuls on a zeroed tile warm the PE HAM clock gate
    (1.2 -> 2.4 GHz); the first real matmuls' DMA deps are minimized
    (w ch0 split per-tap-triple on the Scalar ring, x rows 0-8 first on
    the Sync ring) and half-round 0 runs taps 0-2 across both groups
    before taps 3-8 so compute never outruns the staged chunks.
"""

import numpy as np

B, CIN, COUT, H, W = 32, 128, 256, 56, 56
NCORES = 8
BL = B // NCORES          # images per core
HP = H + 2                # padded height/width (58)
NPIX = H * W              # 3136
NPAD = HP * HP            # 3364
SCALE = 4096.0
OSCALE = 1.0 / (SCALE * SCALE)
GROUP_ROWS = 7            # output rows per PSUM bank
GRP_PIX = GROUP_ROWS * W  # 392
HR_PIX = 2 * GRP_PIX      # 784 px per half-round (2 banks)
ROUND_PIX = 4 * GRP_PIX   # 1568 px per (ch, half) round
NWARM = 32                # dummy matmuls to warm the PE clock gate

_cache = {}


def _build():
    import concourse.bacc as bacc
    import concourse.mybir as mybir
    import concourse.tile as tile

    f32, f16 = mybir.dt.float32, mybir.dt.float16
    Copy = mybir.ActivationFunctionType.Copy

    nc = bacc.Bacc("TRN2", target_bir_lowering=False)
    # x arrives zero-padded to 58x58 and pre-quantized to fp16 on the host
    x_in = nc.dram_tensor("x", [BL, CIN, NPAD], f16, kind="ExternalInput")
    w_in = nc.dram_tensor("w", [CIN, 9 * COUT], f16, kind="ExternalInput")
    out = nc.dram_tensor("out", [BL, COUT, NPIX], f16, kind="ExternalOutput")

    with tile.TileContext(nc) as tc:
        with (
            tc.tile_pool(name="fixed", bufs=1) as fx,
            tc.tile_pool(name="psum", bufs=1, space="PSUM") as pp,
        ):
            xhs = [fx.tile([CIN, NPAD], f16, name=f"xh{i}") for i in range(2)]
            osbs = [fx.tile([128, HR_PIX], f16, name=f"osb{i}") for i in range(4)]
            pq = [pp.tile([128, 2 * 512], f32, name=f"pq{i}") for i in range(4)]
            w16 = fx.tile([CIN, 9 * COUT], f16)
            dummy = fx.tile([128, 256], f16, name="dummy")

            # ---- PE warm-up: keep the HAM activity window busy from engine
            # start so the real matmul stream begins at 2.4 GHz.
            nc.gpsimd.memset(dummy[:], 0.0)
            for i in range(NWARM):
                nc.tensor.matmul(
                    pq[0][:, :128], dummy[:, :128], dummy[:, 128:],
                    start=True, stop=True,
                )

            def stage(b, r0, r1):
                lo, hi = r0 * HP, r1 * HP
                nc.sync.dma_start(out=xhs[b % 2][:, lo:hi], in_=x_in[b, :, lo:hi])

            # x chunks ride the Sync HW-DGE ring, w chunks the Scalar ring:
            # both first transfers issue concurrently.
            HW_COLS = 9 * 128  # 1152 columns per cout-half
            stage(0, 0, 9)
            nc.scalar.dma_start(out=w16[:, : 3 * 128], in_=w_in[:, : 3 * 128])
            stage(0, 9, 16)
            nc.scalar.dma_start(out=w16[:, 3 * 128 : 6 * 128], in_=w_in[:, 3 * 128 : 6 * 128])
            stage(0, 16, 30)
            nc.scalar.dma_start(out=w16[:, 6 * 128 : HW_COLS], in_=w_in[:, 6 * 128 : HW_COLS])
            stage(0, 30, HP)
            nc.scalar.dma_start(out=w16[:, HW_COLS:], in_=w_in[:, HW_COLS:])
            stage(1, 0, HP)

            hr = 0
            for b in range(BL):
                if b >= 2:
                    stage(b, 0, HP)
                xh3 = xhs[b % 2][:].rearrange("p (h w) -> p h w", h=HP)

                for ch in range(2):
                    for half in range(2):
                        for hi in range(2):
                            t = pq[hr % 4]
                            osb = osbs[hr % 4]
                            gs = (2 * hi, 2 * hi + 1)
                            if hr == 0:
                                # tap-triple major: each triple needs only one
                                # staged w chunk and the already-landed x rows,
                                # so the cold ramp never stalls on DMA
                                order = [
                                    (tap, s)
                                    for tri in range(3)
                                    for s in range(2)
                                    for tap in range(3 * tri, 3 * tri + 3)
                                ]
                            else:
                                order = [(tap, s) for tap in range(9) for s in range(2)]
                            for tap, s in order:
                                g = gs[s]
                                dh, dw = divmod(tap, 3)
                                wsl = w16[:, ch * HW_COLS + tap * 128 : ch * HW_COLS + tap * 128 + 128]
                                r0 = (half * 4 + g) * GROUP_ROWS
                                mv = xh3[:, r0 + dh : r0 + dh + GROUP_ROWS, dw : dw + W]
                                nc.tensor.matmul(
                                    t[:, s * 512 : s * 512 + GRP_PIX], wsl, mv,
                                    start=(tap == 0), stop=(tap == 8),
                                )
                            base = half * ROUND_PIX + 2 * hi * GRP_PIX
                            if hr == 31:
                                # final half-round: evict the two banks on
                                # both engines in parallel and store on both
                                # DGE rings so the tail is as short as
                                # possible before the fixed epilogue
                                nc.scalar.activation(
                                    osb[:, :GRP_PIX], t[:, :GRP_PIX], Copy, scale=OSCALE
                                )
                                nc.sync.dma_start(
                                    out=out[b, ch * 128 : (ch + 1) * 128, base : base + GRP_PIX],
                                    in_=osb[:, :GRP_PIX],
                                )
                                nc.vector.tensor_scalar_mul(
                                    osb[:, GRP_PIX:], t[:, 512 : 512 + GRP_PIX], OSCALE
                                )
                                nc.scalar.dma_start(
                                    out=out[b, ch * 128 : (ch + 1) * 128, base + GRP_PIX : base + HR_PIX],
                                    in_=osb[:, GRP_PIX:],
                                )
                            else:
                                # one strided 2-bank eviction (scale + f32->f16)
                                src = t[:].rearrange("p (g c) -> p g c", c=512)[:, :, :GRP_PIX]
                                dst = osb[:].rearrange("p (g c) -> p g c", c=GRP_PIX)
                                if hr % 2 == 0:
                                    nc.scalar.activation(dst, src, Copy, scale=OSCALE)
                                else:
                                    nc.vector.tensor_scalar_mul(dst, src, OSCALE)
                                dstap = out[b, ch * 128 : (ch + 1) * 128, base : base + HR_PIX]
                                if hr % 2 == 0:
                                    nc.sync.dma_start(out=dstap, in_=osb[:])
                                else:
                                    nc.scalar.dma_start(out=dstap, in_=osb[:])
                            hr += 1
    nc.compile()
    return nc


def _get_nc():
    if "nc" not in _cache:
        _cache["nc"] = _build()
    return _cache["nc"]


def _maybe_install_trace_bridge():
    """Optional: bridge antenv.axon_hooks so trace=True can capture NTFF."""
    import sys
    import types

    if "antenv.axon_hooks" in sys.modules:
        return
    try:
        from trn_agent_boot.trn_boot import _ntff_profile_via_ctypes

        hook = _ntff_profile_via_ctypes("/opt/axon/libaxon_pjrt.so")
        mod = types.ModuleType("antenv.axon_hooks")
        mod.get_axon_ntff_profile_hook = lambda: hook
        mod.set_axon_ntff_profile_hook = lambda h: None
        import antenv

        sys.modules["antenv.axon_hooks"] = mod
        antenv.axon_hooks = mod
    except Exception:
        pass


def kernel(**inputs):
    import os

    from concourse.bass_utils import run_bass_kernel_spmd

    x = np.asarray(inputs["x"], dtype=np.float32)
    weight = np.asarray(inputs["weight"], dtype=np.float32)
    assert x.shape == (B, CIN, H, W), x.shape
    assert weight.shape == (COUT, CIN, 3, 3), weight.shape

    # rw = round(w*4096) is an integer < 2048 -> exact in fp16.
    # [Cout, Cin, kh, kw] -> [Cin, (ch, kh kw, co128)] so each (ch, tap)
    # slice is a ready [K=ci, M=co] stationary operand, ch-major so the
    # kernel can stage the ch=0 half first.
    rw = np.rint(weight * np.float32(SCALE))
    w_r = np.ascontiguousarray(
        rw.reshape(2, 128, CIN, 9)
        .transpose(2, 0, 3, 1)
        .reshape(CIN, 9 * COUT)
        .astype(np.float16)
    )
    # xh = fp16(x*4096): the *4096 is exact in f32 (power of two), the fp16
    # cast is the only rounding. Zero-pad to 58x58 so every DMA is contiguous.
    xp = np.zeros((B, CIN, HP, HP), dtype=np.float16)
    xp[:, :, 1 : 1 + H, 1 : 1 + W] = (x * np.float32(SCALE)).astype(np.float16)
    xp = xp.reshape(B, CIN, NPAD)
    in_maps = [
        {"x": xp[i * BL : (i + 1) * BL], "w": w_r}
        for i in range(NCORES)
    ]

    trace = bool(int(os.environ.get("KERNEL_TRACE", "0")))
    if trace:
        _maybe_install_trace_bridge()
    nc = _get_nc()
    res = run_bass_kernel_spmd(nc, in_maps, core_ids=list(range(NCORES)), trace=trace)
    _cache["exec_time_ns"] = res.exec_time_ns
    _cache["res"] = res

    outs = [
        res.results[i]["out"].astype(np.float32).reshape(BL, COUT, H, W)
        for i in range(NCORES)
    ]
    return np.concatenate(outs, axis=0)


# revision 15
# speedup vs baseline: 1.0312x; 1.0075x over previous
"""Trainium2 Bass kernel for quantized 3x3 conv2d (stride 1, pad 1).

Reference computes: conv2d(quant16(x), quant16(w)) where quant16 rounds to
signed 16-bit fixed point with 12 fractional bits (round-half-even, /4096).

Strategy (per core, data-parallel over batch: 4 images/core on 8 cores):
  - Tolerance is rel_err < 2e-2; a single fp16 term is plenty (measured
    2.1e-4 on the real inputs): xh = fp16(x*4096) keeps an 11-bit
    significand and rw = round(w*4096) is fp16-exact (|rw| < 2048). Both
    conversions happen on the host; input DMA bytes are halved. The output
    is stored as fp16 too (adds ~5e-4 rel err, total ~7e-4), halving the
    store traffic so the tail never backs up on HBM write bandwidth.
  - 3x3 conv = 9 shifted matmuls accumulating in PSUM over a zero-padded
    58x58 image laid out [Cin=128 partitions, 58*58]. Contraction dim =
    partition dim = Cin = 128. Cout=256 -> two 128-row output chunks.
  - Work is cut into 32 half-rounds of 2 row-groups (2 PSUM banks, 784 px)
    cycling over four 2-bank PSUM tiles. Each half-round: 18 matmuls
    (taps outer, 2 share a stationary), then ONE strided 2-bank eviction
    (alternating ScalarE/VectorE, applying the 2^-24 fixed-point scale and
    the f32->f16 convert) and ONE 200KB store (alternating HW-DGE rings).
    Separate tiles keep evictions off the matmul critical path (the
    dependency tracker is whole-tile), and the small evict+store tail after
    the last matmul hides under the fixed ~8us semaphore-cleanup epilogue.
  - Ramp: ~32 dummy matmuls on a zeroed tile warm the PE HAM clock gate
    (1.2 -> 2.4 GHz); the first real matmuls' DMA deps are minimized
    (w ch0 split per-tap-triple on the Scalar ring, x rows 0-8 first on
    the Sync ring) and half-round 0 runs taps 0-2 across both groups
    before taps 3-8 so compute never outruns the staged chunks.
"""

import numpy as np

B, CIN, COUT, H, W = 32, 128, 256, 56, 56
NCORES = 8
BL = B // NCORES          # images per core
HP = H + 2                # padded height/width (58)
NPIX = H * W              # 3136
NPAD = HP * HP            # 3364
SCALE = 4096.0
OSCALE = 1.0 / (SCALE * SCALE)
GROUP_ROWS = 7            # output rows per PSUM bank
GRP_PIX = GROUP_ROWS * W  # 392
HR_PIX = 2 * GRP_PIX      # 784 px per half-round (2 banks)
ROUND_PIX = 4 * GRP_PIX   # 1568 px per (ch, half) round
NWARM = 32                # dummy matmuls to warm the PE clock gate

_cache = {}


def _build():
    import concourse.bacc as bacc
    import concourse.mybir as mybir
    import concourse.tile as tile

    f32, f16 = mybir.dt.float32, mybir.dt.float16
    Copy = mybir.ActivationFunctionType.Copy

    nc = bacc.Bacc("TRN2", target_bir_lowering=False)
    # x arrives zero-padded to 58x58 and pre-quantized to fp16 on the host
    x_in = nc.dram_tensor("x", [BL, CIN, NPAD], f16, kind="ExternalInput")
    w_in = nc.dram_tensor("w", [CIN, 9 * COUT], f16, kind="ExternalInput")
    out = nc.dram_tensor("out", [BL, COUT, NPIX], f16, kind="ExternalOutput")

    with tile.TileContext(nc) as tc:
        with (
            tc.tile_pool(name="fixed", bufs=1) as fx,
            tc.tile_pool(name="psum", bufs=1, space="PSUM") as pp,
        ):
            xhs = [fx.tile([CIN, NPAD], f16, name=f"xh{i}") for i in range(2)]
            osbs = [fx.tile([128, HR_PIX], f16, name=f"osb{i}") for i in range(4)]
            pq = [pp.tile([128, 2 * 512], f32, name=f"pq{i}") for i in range(4)]
            w16 = fx.tile([CIN, 9 * COUT], f16)
            dummy = fx.tile([128, 256], f16, name="dummy")

            # ---- PE warm-up: keep the HAM activity window busy from engine
            # start so the real matmul stream begins at 2.4 GHz.
            nc.gpsimd.memset(dummy[:], 0.0)
            for i in range(NWARM):
                nc.tensor.matmul(
                    pq[0][:, :128], dummy[:, :128], dummy[:, 128:],
                    start=True, stop=True,
                )

            def stage(b, r0, r1):
                lo, hi = r0 * HP, r1 * HP
                nc.sync.dma_start(out=xhs[b % 2][:, lo:hi], in_=x_in[b, :, lo:hi])

            # x chunks ride the Sync HW-DGE ring, w chunks the Scalar ring:
            # both first transfers issue concurrently.
            HW_COLS = 9 * 128  # 1152 columns per cout-half
            stage(0, 0, 9)
            nc.scalar.dma_start(out=w16[:, : 3 * 128], in_=w_in[:, : 3 * 128])
            stage(0, 9, 16)
            nc.scalar.dma_start(out=w16[:, 3 * 128 : 6 * 128], in_=w_in[:, 3 * 128 : 6 * 128])
            stage(0, 16, 30)
            nc.scalar.dma_start(out=w16[:, 6 * 128 : HW_COLS], in_=w_in[:, 6 * 128 : HW_COLS])
            stage(0, 30, HP)
            nc.scalar.dma_start(out=w16[:, HW_COLS:], in_=w_in[:, HW_COLS:])
            stage(1, 0, HP)

            hr = 0
            for b in range(BL):
                if b >= 2:
                    stage(b, 0, HP)
                xh3 = xhs[b % 2][:].rearrange("p (h w) -> p h w", h=HP)

                for ch in range(2):
                    for half in range(2):
                        for hi in range(2):
                            t = pq[hr % 4]
                            osb = osbs[hr % 4]
                            gs = (2 * hi, 2 * hi + 1)
                            if hr == 0:
                                # tap-triple major: each triple needs only one
                                # staged w chunk and the already-landed x rows,
                                # so the cold ramp never stalls on DMA
                                order = [
                                    (tap, s)
                                    for tri in range(3)
                                    for s in range(2)
                                    for tap in range(3 * tri, 3 * tri + 3)
                                ]
                            else:
                                order = [(tap, s) for tap in range(9) for s in range(2)]
                            for tap, s in order:
                                g = gs[s]
                                dh, dw = divmod(tap, 3)
                                wsl = w16[:, ch * HW_COLS + tap * 128 : ch * HW_COLS + tap * 128 + 128]
                                r0 = (half * 4 + g) * GROUP_ROWS
                                mv = xh3[:, r0 + dh : r0 + dh + GROUP_ROWS, dw : dw + W]
                                nc.tensor.matmul(
                                    t[:, s * 512 : s * 512 + GRP_PIX], wsl, mv,
                                    start=(tap == 0), stop=(tap == 8),
                                )
                            base = half * ROUND_PIX + 2 * hi * GRP_PIX
                            if hr == 31:
                                # final half-round: evict the two banks on
                                # both engines in parallel and store on both
                                # DGE rings so the tail is as short as
                                # possible before the fixed epilogue
                                nc.scalar.activation(
                                    osb[:, :GRP_PIX], t[:, :GRP_PIX], Copy, scale=OSCALE
                                )
                                nc.sync.dma_start(
                                    out=out[b, ch * 128 : (ch + 1) * 128, base : base + GRP_PIX],
                                    in_=osb[:, :GRP_PIX],
                                )
                                nc.vector.tensor_scalar_mul(
                                    osb[:, GRP_PIX:], t[:, 512 : 512 + GRP_PIX], OSCALE
                                )
                                nc.scalar.dma_start(
                                    out=out[b, ch * 128 : (ch + 1) * 128, base + GRP_PIX : base + HR_PIX],
                                    in_=osb[:, GRP_PIX:],
                                )
                            else:
                                # one strided 2-bank eviction (scale + f32->f16)
                                src = t[:].rearrange("p (g c) -> p g c", c=512)[:, :, :GRP_PIX]
                                dst = osb[:].rearrange("p (g c) -> p g c", c=GRP_PIX)
                                if hr % 2 == 0:
                                    nc.scalar.activation(dst, src, Copy, scale=OSCALE)
                                else:
                                    nc.vector.tensor_scalar_mul(dst, src, OSCALE)
                                dstap = out[b, ch * 128 : (ch + 1) * 128, base : base + HR_PIX]
                                if hr % 2 == 0:
                                    nc.sync.dma_start(out=dstap, in_=osb[:])
                                else:
                                    nc.scalar.dma_start(out=dstap, in_=osb[:])
                            hr += 1
    nc.compile()
    return nc


def _get_nc():
    if "nc" not in _cache:
        _cache["nc"] = _build()
    return _cache["nc"]


def _maybe_install_trace_bridge():
    """Optional: bridge antenv.axon_hooks so trace=True can capture NTFF."""
    import sys
    import types

    if "antenv.axon_hooks" in sys.modules:
        return
    try:
        from trn_agent_boot.trn_boot import _ntff_profile_via_ctypes

        hook = _ntff_profile_via_ctypes("/opt/axon/libaxon_pjrt.so")
        mod = types.ModuleType("antenv.axon_hooks")
        mod.get_axon_ntff_profile_hook = lambda: hook
        mod.set_axon_ntff_profile_hook = lambda h: None
        import antenv

        sys.modules["antenv.axon_hooks"] = mod
        antenv.axon_hooks = mod
    except Exception:
        pass


def kernel(**inputs):
    import os

    from concourse.bass_utils import run_bass_kernel_spmd

    x = np.asarray(inputs["x"], dtype=np.float32)
    weight = np.asarray(inputs["weight"], dtype=np.float32)
    assert x.shape == (B, CIN, H, W), x.shape
    assert weight.shape == (COUT, CIN, 3, 3), weight.shape

    # rw = round(w*4096) is an integer < 2048 -> exact in fp16.
    # [Cout, Cin, kh, kw] -> [Cin, (ch, kh kw, co128)] so each (ch, tap)
    # slice is a ready [K=ci, M=co] stationary operand, ch-major so the
    # kernel can stage the ch=0 half first.
    rw = np.rint(weight * np.float32(SCALE))
    w_r = np.ascontiguousarray(
        rw.reshape(2, 128, CIN, 9)
        .transpose(2, 0, 3, 1)
        .reshape(CIN, 9 * COUT)
        .astype(np.float16)
    )
    # xh = fp16(x*4096): the *4096 is exact in f32 (power of two), the fp16
    # cast is the only rounding. Zero-pad to 58x58 so every DMA is contiguous.
    xp = np.zeros((B, CIN, HP, HP), dtype=np.float16)
    xp[:, :, 1 : 1 + H, 1 : 1 + W] = (x * np.float32(SCALE)).astype(np.float16)
    xp = xp.reshape(B, CIN, NPAD)
    in_maps = [
        {"x": xp[i * BL : (i + 1) * BL], "w": w_r}
        for i in range(NCORES)
    ]

    trace = bool(int(os.environ.get("KERNEL_TRACE", "0")))
    if trace:
        _maybe_install_trace_bridge()
    nc = _get_nc()
    res = run_bass_kernel_spmd(nc, in_maps, core_ids=list(range(NCORES)), trace=trace)
    _cache["exec_time_ns"] = res.exec_time_ns
    _cache["res"] = res

    outs = [
        res.results[i]["out"].astype(np.float32).reshape(BL, COUT, H, W)
        for i in range(NCORES)
    ]
    return np.concatenate(outs, axis=0)


# revision 16
# speedup vs baseline: 1.0330x; 1.0017x over previous
"""Trainium2 Bass kernel for quantized 3x3 conv2d (stride 1, pad 1).

Reference computes: conv2d(quant16(x), quant16(w)) where quant16 rounds to
signed 16-bit fixed point with 12 fractional bits (round-half-even, /4096).

Strategy (per core, data-parallel over batch: 4 images/core on 8 cores):
  - Tolerance is rel_err < 2e-2; a single fp16 term is plenty (measured
    2.1e-4 on the real inputs): xh = fp16(x*4096) keeps an 11-bit
    significand and rw = round(w*4096) is fp16-exact (|rw| < 2048). Both
    conversions happen on the host; input DMA bytes are halved. The output
    is stored as fp16 too (adds ~5e-4 rel err, total ~7e-4), halving the
    store traffic so the tail never backs up on HBM write bandwidth.
  - 3x3 conv = 9 shifted matmuls accumulating in PSUM over a zero-padded
    58x58 image laid out [Cin=128 partitions, 58*58]. Contraction dim =
    partition dim = Cin = 128. Cout=256 -> two 128-row output chunks.
  - Work is cut into 32 half-rounds of 2 row-groups (2 PSUM banks, 784 px)
    cycling over four 2-bank PSUM tiles. Each half-round: 18 matmuls
    (taps outer, 2 share a stationary), then ONE strided 2-bank eviction
    (alternating ScalarE/VectorE, applying the 2^-24 fixed-point scale and
    the f32->f16 convert) and ONE 200KB store (alternating HW-DGE rings).
    Separate tiles keep evictions off the matmul critical path (the
    dependency tracker is whole-tile), and the small evict+store tail after
    the last matmul hides under the fixed ~8us semaphore-cleanup epilogue.
  - Ramp: ~32 dummy matmuls on a zeroed tile warm the PE HAM clock gate
    (1.2 -> 2.4 GHz); the first real matmuls' DMA deps are minimized
    (w ch0 split per-tap-triple on the Scalar ring, x rows 0-8 first on
    the Sync ring) and half-round 0 runs taps 0-2 across both groups
    before taps 3-8 so compute never outruns the staged chunks.
"""

import numpy as np

B, CIN, COUT, H, W = 32, 128, 256, 56, 56
NCORES = 8
BL = B // NCORES          # images per core
HP = H + 2                # padded height/width (58)
NPIX = H * W              # 3136
NPAD = HP * HP            # 3364
SCALE = 4096.0
OSCALE = 1.0 / (SCALE * SCALE)
GROUP_ROWS = 7            # output rows per PSUM bank
GRP_PIX = GROUP_ROWS * W  # 392
HR_PIX = 2 * GRP_PIX      # 784 px per half-round (2 banks)
ROUND_PIX = 4 * GRP_PIX   # 1568 px per (ch, half) round
NWARM = 32                # dummy matmuls to warm the PE clock gate

_cache = {}


def _build():
    import concourse.bacc as bacc
    import concourse.mybir as mybir
    import concourse.tile as tile

    f32, f16 = mybir.dt.float32, mybir.dt.float16
    Copy = mybir.ActivationFunctionType.Copy

    nc = bacc.Bacc("TRN2", target_bir_lowering=False)
    # x arrives zero-padded to 58x58 and pre-quantized to fp16 on the host
    x_in = nc.dram_tensor("x", [BL, CIN, NPAD], f16, kind="ExternalInput")
    w_in = nc.dram_tensor("w", [CIN, 9 * COUT], f16, kind="ExternalInput")
    out = nc.dram_tensor("out", [BL, COUT, NPIX], f16, kind="ExternalOutput")

    with tile.TileContext(nc) as tc:
        with (
            tc.tile_pool(name="fixed", bufs=1) as fx,
            tc.tile_pool(name="psum", bufs=1, space="PSUM") as pp,
        ):
            xhs = [fx.tile([CIN, NPAD], f16, name=f"xh{i}") for i in range(2)]
            osbs = [fx.tile([128, HR_PIX], f16, name=f"osb{i}") for i in range(4)]
            pq = [pp.tile([128, 2 * 512], f32, name=f"pq{i}") for i in range(4)]
            w16 = fx.tile([CIN, 9 * COUT], f16)
            dummy = fx.tile([128, 256], f16, name="dummy")

            # ---- PE warm-up: keep the HAM activity window busy from engine
            # start so the real matmul stream begins at 2.4 GHz.
            nc.gpsimd.memset(dummy[:], 0.0)
            for i in range(NWARM):
                nc.tensor.matmul(
                    pq[0][:, :128], dummy[:, :128], dummy[:, 128:],
                    start=True, stop=True,
                )

            def stage(b, r0, r1):
                lo, hi = r0 * HP, r1 * HP
                nc.sync.dma_start(out=xhs[b % 2][:, lo:hi], in_=x_in[b, :, lo:hi])

            # x chunks ride the Sync HW-DGE ring, w chunks the Scalar ring:
            # both first transfers issue concurrently.
            HW_COLS = 9 * 128  # 1152 columns per cout-half
            stage(0, 0, 9)
            nc.scalar.dma_start(out=w16[:, : 3 * 128], in_=w_in[:, : 3 * 128])
            stage(0, 9, 16)
            nc.scalar.dma_start(out=w16[:, 3 * 128 : 6 * 128], in_=w_in[:, 3 * 128 : 6 * 128])
            stage(0, 16, 30)
            nc.scalar.dma_start(out=w16[:, 6 * 128 : HW_COLS], in_=w_in[:, 6 * 128 : HW_COLS])
            stage(0, 30, HP)
            nc.scalar.dma_start(out=w16[:, HW_COLS:], in_=w_in[:, HW_COLS:])
            stage(1, 0, HP)

            hr = 0
            for b in range(BL):
                if b >= 2:
                    stage(b, 0, HP)
                xh3 = xhs[b % 2][:].rearrange("p (h w) -> p h w", h=HP)

                for ch in range(2):
                    for half in range(2):
                        for hi in range(2):
                            t = pq[hr % 4]
                            osb = osbs[hr % 4]
                            gs = (2 * hi, 2 * hi + 1)
                            if hr == 0:
                                # tap-triple major: each triple needs only one
                                # staged w chunk and the already-landed x rows,
                                # so the cold ramp never stalls on DMA
                                order = [
                                    (tap, s)
                                    for tri in range(3)
                                    for s in range(2)
                                    for tap in range(3 * tri, 3 * tri + 3)
                                ]
                            else:
                                order = [(tap, s) for tap in range(9) for s in range(2)]
                            for tap, s in order:
                                g = gs[s]
                                dh, dw = divmod(tap, 3)
                                wsl = w16[:, ch * HW_COLS + tap * 128 : ch * HW_COLS + tap * 128 + 128]
                                r0 = (half * 4 + g) * GROUP_ROWS
                                mv = xh3[:, r0 + dh : r0 + dh + GROUP_ROWS, dw : dw + W]
                                nc.tensor.matmul(
                                    t[:, s * 512 : s * 512 + GRP_PIX], wsl, mv,
                                    start=(tap == 0), stop=(tap == 8),
                                )
                            base = half * ROUND_PIX + 2 * hi * GRP_PIX
                            if hr == 31:
                                # final half-round: evict the two banks on
                                # both engines in parallel and store on both
                                # DGE rings so the tail is as short as
                                # possible before the fixed epilogue
                                nc.scalar.activation(
                                    osb[:, :GRP_PIX], t[:, :GRP_PIX], Copy, scale=OSCALE
                                )
                                nc.sync.dma_start(
                                    out=out[b, ch * 128 : (ch + 1) * 128, base : base + GRP_PIX],
                                    in_=osb[:, :GRP_PIX],
                                )
                                nc.vector.tensor_scalar_mul(
                                    osb[:, GRP_PIX:], t[:, 512 : 512 + GRP_PIX], OSCALE
                                )
                                nc.scalar.dma_start(
                                    out=out[b, ch * 128 : (ch + 1) * 128, base + GRP_PIX : base + HR_PIX],
                                    in_=osb[:, GRP_PIX:],
                                )
                            else:
                                # one strided 2-bank eviction (scale + f32->f16)
                                src = t[:].rearrange("p (g c) -> p g c", c=512)[:, :, :GRP_PIX]
                                dst = osb[:].rearrange("p (g c) -> p g c", c=GRP_PIX)
                                if hr % 2 == 0:
                                    nc.scalar.activation(dst, src, Copy, scale=OSCALE)
                                else:
                                    nc.vector.tensor_scalar_mul(dst, src, OSCALE)
                                dstap = out[b, ch * 128 : (ch + 1) * 128, base : base + HR_PIX]
                                if hr % 2 == 0:
                                    nc.sync.dma_start(out=dstap, in_=osb[:])
                                else:
                                    nc.scalar.dma_start(out=dstap, in_=osb[:])
                            hr += 1

    # Drop LDWEIGHTS whose stationary operand is identical to the previous
    # one (two matmuls share each tap's weights): the PE array keeps loaded
    # weights across matmuls, so the re-load is pure NX dispatch overhead.
    # The ramp region (warm-up + first half-round) is left untouched so
    # compile-time wait migration onto shared LDWEIGHTS can't stall it.
    import json as _json

    fn = list(nc.m.functions)[0]
    for blk in fn.blocks:
        if "tile_context" not in blk.name:
            continue
        prev_sig = None
        seen_lw = 0
        to_del = []
        for inst in list(blk.instructions):
            tn = type(inst).__name__
            if tn == "InstLdweights":
                seen_lw += 1
                d = _json.loads(mybir.instruction_to_pretty_json_string(inst))
                d.pop("debug", None)
                d.pop("name", None)
                d.pop("sync_info", None)
                s = _json.dumps(d, sort_keys=True)
                if s == prev_sig and seen_lw > NWARM + 18:
                    to_del.append(inst)
                else:
                    prev_sig = s
            elif tn == "InstMatmult":
                pass
            elif getattr(inst, "engine", None) == mybir.EngineType.PE:
                prev_sig = None
        for inst in to_del:
            blk.instructions.remove(inst)

    nc.compile()
    return nc


def _get_nc():
    if "nc" not in _cache:
        _cache["nc"] = _build()
    return _cache["nc"]


def _maybe_install_trace_bridge():
    """Optional: bridge antenv.axon_hooks so trace=True can capture NTFF."""
    import sys
    import types

    if "antenv.axon_hooks" in sys.modules:
        return
    try:
        from trn_agent_boot.trn_boot import _ntff_profile_via_ctypes

        hook = _ntff_profile_via_ctypes("/opt/axon/libaxon_pjrt.so")
        mod = types.ModuleType("antenv.axon_hooks")
        mod.get_axon_ntff_profile_hook = lambda: hook
        mod.set_axon_ntff_profile_hook = lambda h: None
        import antenv

        sys.modules["antenv.axon_hooks"] = mod
        antenv.axon_hooks = mod
    except Exception:
        pass


def kernel(**inputs):
    import os

    from concourse.bass_utils import run_bass_kernel_spmd

    x = np.asarray(inputs["x"], dtype=np.float32)
    weight = np.asarray(inputs["weight"], dtype=np.float32)
    assert x.shape == (B, CIN, H, W), x.shape
    assert weight.shape == (COUT, CIN, 3, 3), weight.shape

    # rw = round(w*4096) is an integer < 2048 -> exact in fp16.
    # [Cout, Cin, kh, kw] -> [Cin, (ch, kh kw, co128)] so each (ch, tap)
    # slice is a ready [K=ci, M=co] stationary operand, ch-major so the
    # kernel can stage the ch=0 half first.
    rw = np.rint(weight * np.float32(SCALE))
    w_r = np.ascontiguousarray(
        rw.reshape(2, 128, CIN, 9)
        .transpose(2, 0, 3, 1)
        .reshape(CIN, 9 * COUT)
        .astype(np.float16)
    )
    # xh = fp16(x*4096): the *4096 is exact in f32 (power of two), the fp16
    # cast is the only rounding. Zero-pad to 58x58 so every DMA is contiguous.
    xp = np.zeros((B, CIN, HP, HP), dtype=np.float16)
    xp[:, :, 1 : 1 + H, 1 : 1 + W] = (x * np.float32(SCALE)).astype(np.float16)
    xp = xp.reshape(B, CIN, NPAD)
    in_maps = [
        {"x": xp[i * BL : (i + 1) * BL], "w": w_r}
        for i in range(NCORES)
    ]

    trace = bool(int(os.environ.get("KERNEL_TRACE", "0")))
    if trace:
        _maybe_install_trace_bridge()
    nc = _get_nc()
    res = run_bass_kernel_spmd(nc, in_maps, core_ids=list(range(NCORES)), trace=trace)
    _cache["exec_time_ns"] = res.exec_time_ns
    _cache["res"] = res

    outs = [
        res.results[i]["out"].astype(np.float32).reshape(BL, COUT, H, W)
        for i in range(NCORES)
    ]
    return np.concatenate(outs, axis=0)
